# revision 13
# baseline (speedup 1.0000x reference)
"""Trainium2 Bass kernel for nn_Barrier_Net (DeepSet GNN message passing).

Strategy (8 NeuronCores, SPMD):
  - Each core owns 2048 contiguous agents (16 blocks of 128 agents).
  - Host slices the sorted edge list at agent-range boundaries and pads each
    128-agent block to a uniform C chunks of 128 edges (identical program on
    every core; no collectives).
  - phi layer 1 (E x 4 @ 4 x 256, 0.5% of FLOPs) runs on the host; its
    ReLU output ships to the device as fp8e4m3 packed [128, 2, E] (the two
    128-feature halves on the k-axis) so phi layer 2 runs as K=256 fp8
    DoubleRow matmuls (0.5 cycles/row).
  - Weights Wp2/Wp3 use split-fp8 error compensation (W ~ hi + lo, two
    accumulating DoubleRow matmuls): weight quantization error is correlated
    across edges and does not average out in the segment sum, unlike the
    per-edge activation rounding noise.
  - phi layer 3 and the segment-sum (one-hot matmul; one-hot built on host,
    shipped as fp8 0/1) also run in fp8 DoubleRow.  PSUM stays fp32.
  - bp3 folds in as a rank-1 (degree x bp3) matmul; br1 via a constant-ones
    row appended to the aggregate; br2 via a rank-1 (br2 x ones) matmul.
    When bp2 is all-zero (checked at runtime) the h2 PSUM drain is a single
    merged bias-free ReLU instruction per chunk.
  - rho runs data-parallel over the agent dim in fp32r, interleaved with the
    edge blocks (group g is emitted after block 4g+3).
  - Only ACT/DVE can read PSUM (GPSIMD is SBUF-only), so the PSUM drains
    alternate between those two engines, weighted by their clocks.
  - The barrier term and br3 (negligible FLOPs) are added on the host.
"""

import numpy as np

N_AGENTS = 16384
N_EDGES = 524288
N_CORES = 8
AG_PER_CORE = N_AGENTS // N_CORES  # 2048
BLK = 128                          # agents per block
NBLK = AG_PER_CORE // BLK          # 16 blocks per core
MARGIN = 1.2 * 0.15                # barrier margin

A_BUFS = 8
PS_MLP_BUFS = 3
PS_SM_BUFS = 1
PS_AGG_BUFS = 1

_compiled = {}


def _build(C):
    """Build + schedule the SPMD Bass program for C 128-edge subchunks/block."""
    from contextlib import ExitStack

    import concourse.bass as bass
    import concourse.tile as tile
    from concourse import bacc, mybir

    FP = mybir.dt.float32
    RDT = mybir.dt.float32r
    BF = mybir.dt.bfloat16
    F8 = mybir.dt.float8e4
    DR = mybir.MatmulPerfMode.DoubleRow
    E_BLK = C * 128                     # padded edges per block

    nc = bacc.Bacc("TRN2", target_bir_lowering=False, debug=False,
                   num_devices=N_CORES)

    def din(name, shape, dt):
        return nc.dram_tensor(name, shape, dt, kind="ExternalInput").ap()

    h1d = din("h1", [128, NBLK * 2 * E_BLK], F8)
    ohd = din("oh", [128, NBLK * C * 128], BF)
    Wp2 = din("Wp2", [128, 1024], F8)       # hi || lo, each [128, 2, 256]
    Wp3 = din("Wp3", [256, 64], BF)
    bp2 = din("bp2", [256, 1], FP)
    bp3 = din("bp3", [1, 64], BF)
    degT = din("degT", [1, AG_PER_CORE], BF)
    ident = din("ident", [128, 128], FP)
    ones = din("ones", [1, AG_PER_CORE], RDT)
    Wr1 = din("Wr1", [65, 256], RDT)
    Wr2 = din("Wr2", [256, 256], RDT)
    br2 = din("br2", [1, 256], RDT)
    Wr3 = din("Wr3", [256, 2], FP)
    out_d = nc.dram_tensor("out", [128, 32], FP, kind="ExternalOutput").ap()

    RELU = mybir.ActivationFunctionType.Relu
    COPY = mybir.ActivationFunctionType.Copy
    ADD = mybir.AluOpType.add
    MAX = mybir.AluOpType.max

    with tile.TileContext(nc) as tc, ExitStack() as ctx:
        consts = ctx.enter_context(tc.tile_pool(name="consts", bufs=1))
        h1_pool = ctx.enter_context(tc.tile_pool(name="h1p", bufs=2))
        oh_pool = ctx.enter_context(tc.tile_pool(name="ohp", bufs=2))
        a_pool = ctx.enter_context(tc.tile_pool(name="acts", bufs=A_BUFS))
        ps_mlp = ctx.enter_context(
            tc.tile_pool(name="ps_mlp", bufs=PS_MLP_BUFS, space="PSUM"))
        ps_sm = ctx.enter_context(
            tc.tile_pool(name="ps_sm", bufs=PS_SM_BUFS, space="PSUM"))
        ps_agg = ctx.enter_context(
            tc.tile_pool(name="ps_agg", bufs=PS_AGG_BUFS, space="PSUM"))

        def cload(name, ap, shape=None, dt=FP):
            t = consts.tile(shape or list(ap.shape), dt, tag=name, name=name)
            nc.sync.dma_start(t[:], ap)
            return t

        wp2h_s = cload("wp2h", Wp2[:, 0:512], shape=[128, 2, 256], dt=F8)
        wp2l_s = cload("wp2l", Wp2[:, 512:1024], shape=[128, 2, 256], dt=F8)
        wp3a_s = cload("wp3a", Wp3[0:128, :], dt=BF)
        wp3b_s = cload("wp3b", Wp3[128:256, :], dt=BF)
        bp2a = cload("bp2a", bp2[0:128, :])
        bp2b = cload("bp2b", bp2[128:256, :])
        bp3_s = cload("bp3", bp3, dt=BF)
        degT_s = cload("degT", degT, dt=BF)
        ident_s = cload("ident", ident)
        wr1_s = cload("wr1", Wr1, dt=RDT)
        wr2a_s = cload("wr2a", Wr2[0:128, :], dt=RDT)
        wr2b_s = cload("wr2b", Wr2[128:256, :], dt=RDT)
        br2_s = cload("br2", br2, dt=RDT)
        wr3a_s = cload("wr3a", Wr3[0:128, :])
        wr3b_s = cload("wr3b", Wr3[128:256, :])
        ones_s = cload("ones_s", ones, dt=RDT)
        aggT_s = consts.tile([65, AG_PER_CORE], RDT, tag="aggT", name="aggT")
        nc.sync.dma_start(aggT_s[64:65, :], ones)
        osb = consts.tile([128, 32], FP, tag="osb", name="osb")

        # elementwise helpers; only ACT(0)/DVE(1) may read PSUM ------------
        def e_relu(eng, out, in_, bias=None):
            if eng == 0:
                nc.scalar.activation(out, in_, RELU,
                                     bias=0.0 if bias is None else bias)
            else:
                if bias is None:
                    nc.vector.tensor_scalar(out, in_, 0.0, None, MAX)
                else:
                    nc.vector.tensor_scalar(out, in_, bias, 0.0, ADD, MAX)

        def e_copy(eng, out, in_):
            if eng == 0:
                nc.scalar.activation(out, in_, COPY)
            else:
                nc.vector.tensor_copy(out, in_)

        # (ea, eb, ecp) rotation per chunk: the two h2 halves drain on both
        # engines concurrently (shorter critical path); copies lean ACT.
        ROT3 = [(0, 1, 0), (1, 0, 0), (0, 1, 1)]

        # chunk sizes per block: C//4 chunks of 512 edges + one tail
        sizes = [512] * (C // 4)
        if C % 4:
            sizes.append(128 * (C % 4))

        def rho_group(g):
            sl = slice(g * 512, (g + 1) * 512)
            pr1 = ps_mlp.tile([128, 2, 512], FP, tag="psmlp")
            nc.tensor.matmul(pr1[:, 0, :], wr1_s[:, 0:128], aggT_s[:, sl],
                             start=True, stop=True)
            nc.tensor.matmul(pr1[:, 1, :], wr1_s[:, 128:256], aggT_s[:, sl],
                             start=True, stop=True)
            r1 = a_pool.tile([128, 2, 512], RDT, tag="big")
            e_relu(0, r1[:, 0, :], pr1[:, 0, :])
            e_relu(1, r1[:, 1, :], pr1[:, 1, :])
            pr2 = ps_mlp.tile([128, 2, 512], FP, tag="psmlp")
            nc.tensor.matmul(pr2[:, 0, :], wr2a_s[:, 0:128], r1[:, 0, :],
                             start=True, stop=False)
            nc.tensor.matmul(pr2[:, 0, :], wr2b_s[:, 0:128], r1[:, 1, :],
                             start=False, stop=False)
            nc.tensor.matmul(pr2[:, 0, :], br2_s[0:1, 0:128],
                             ones_s[0:1, sl], start=False, stop=True)
            nc.tensor.matmul(pr2[:, 1, :], wr2a_s[:, 128:256], r1[:, 0, :],
                             start=True, stop=False)
            nc.tensor.matmul(pr2[:, 1, :], wr2b_s[:, 128:256], r1[:, 1, :],
                             start=False, stop=False)
            nc.tensor.matmul(pr2[:, 1, :], br2_s[0:1, 128:256],
                             ones_s[0:1, sl], start=False, stop=True)
            r2 = a_pool.tile([128, 2, 512], FP, tag="big")
            e_relu(g % 2, r2[:, 0, :], pr2[:, 0, :])
            e_relu(1 - g % 2, r2[:, 1, :], pr2[:, 1, :])
            pso = ps_sm.tile([128, 8], FP, tag="sm")
            for s in range(4):
                ssl = slice(s * 128, (s + 1) * 128)
                nc.tensor.matmul(pso[:, s * 2:s * 2 + 2], r2[:, 0, ssl],
                                 wr3a_s[:], start=True, stop=False)
                nc.tensor.matmul(pso[:, s * 2:s * 2 + 2], r2[:, 1, ssl],
                                 wr3b_s[:], start=False, stop=True)
            e_copy(g % 2, osb[:, g * 8:(g + 1) * 8], pso[:])

        cidx = 0
        for j in range(NBLK):
            h1_blk = h1_pool.tile([128, 2, E_BLK], F8, tag="h1")
            nc.sync.dma_start(h1_blk[:],
                              h1d[:, j * 2 * E_BLK:(j + 1) * 2 * E_BLK])
            oh_blk = oh_pool.tile([128, C, 128], BF, tag="oh")
            nc.sync.dma_start(oh_blk[:],
                              ohd[:, j * C * 128:(j + 1) * C * 128])
            pagg = ps_agg.tile([128, 64], FP, tag="agg")
            first = True
            off = 0
            s0 = 0
            for n in sizes:
                nsub = n // 128
                ea, eb, ce = ROT3[cidx % 3]
                cidx += 1
                h1sl = h1_blk[:, :, off:off + n]
                ps2 = ps_mlp.tile([128, 2, 512], FP, tag="psmlp")
                nc.tensor.matmul(ps2[:, 0, 0:n], wp2h_s[:, :, 0:128], h1sl,
                                 start=True, stop=False, perf_mode=DR)
                nc.tensor.matmul(ps2[:, 0, 0:n], wp2l_s[:, :, 0:128], h1sl,
                                 start=False, stop=True, perf_mode=DR)
                nc.tensor.matmul(ps2[:, 1, 0:n], wp2h_s[:, :, 128:256], h1sl,
                                 start=True, stop=False, perf_mode=DR)
                nc.tensor.matmul(ps2[:, 1, 0:n], wp2l_s[:, :, 128:256], h1sl,
                                 start=False, stop=True, perf_mode=DR)
                h2 = a_pool.tile([128, 2, 512], BF, tag="h2")
                e_relu(ea, h2[:, 0, 0:n], ps2[:, 0, 0:n], bias=bp2a[:, 0:1])
                e_relu(eb, h2[:, 1, 0:n], ps2[:, 1, 0:n], bias=bp2b[:, 0:1])
                ps3 = ps_sm.tile([128, 4, 64], FP, tag="sm")
                for s in range(nsub):
                    ssl = slice(s * 128, (s + 1) * 128)
                    nc.tensor.matmul(ps3[:, s, :], h2[:, 0, ssl], wp3a_s[:],
                                     start=True, stop=False)
                    nc.tensor.matmul(ps3[:, s, :], h2[:, 1, ssl], wp3b_s[:],
                                     start=False, stop=True)
                h3 = a_pool.tile([128, 4, 64], BF, tag="h3")
                e_copy(ce, h3[:, 0:nsub, :], ps3[:, 0:nsub, :])
                for t in range(nsub):
                    nc.tensor.matmul(pagg[:], oh_blk[:, s0 + t, :],
                                     h3[:, t, :], start=first, stop=False)
                    first = False
                s0 += nsub
                off += n
            # fold in bp3: agg += deg (x) bp3   (rank-1)
            nc.tensor.matmul(pagg[:], degT_s[:, j * 128:(j + 1) * 128],
                             bp3_s[:], start=first, stop=True)
            agg_sb = a_pool.tile([128, 64], FP, tag="aggsb")
            e_copy(1, agg_sb[:], pagg[:])
            pst = ps_sm.tile([64, 128], FP, tag="sm")
            nc.tensor.transpose(pst[:], agg_sb[:], ident_s[:])
            e_copy(1, aggT_s[0:64, j * 128:(j + 1) * 128], pst[:])
            if j % 4 == 3:
                rho_group(j // 4)

        nc.sync.dma_start(out_d, osb[:])

    nc.compile()
    return nc


def _split_fp8(W):
    import ml_dtypes
    F8 = ml_dtypes.float8_e4m3
    hi = W.astype(F8)
    lo = (W - hi.astype(np.float32)).astype(F8)
    return hi, lo


def _prep_inputs(edge_feats, segment_ids, ws):
    """Host-side: phi layer 1, one-hot build, shard + pad.

    Returns (C, in_maps)."""
    import ml_dtypes
    F8 = ml_dtypes.float8_e4m3

    seg = np.asarray(segment_ids).astype(np.int64)
    ef = np.asarray(edge_feats, dtype=np.float32)
    bounds = np.searchsorted(seg, np.arange(0, N_AGENTS + 1, BLK))
    counts = np.diff(bounds)                      # edges per 128-agent block
    C = int(np.ceil(counts.max() / 128))
    E_BLK = C * 128

    # phi layer 1 on host (0.5% of FLOPs): h1 = relu(x @ Wp1 + bp1)
    W1, b1 = ws["Wp1"], ws["bp1"]
    h1 = ef[:, 0:1] * W1[0] + ef[:, 1:2] * W1[1]
    h1 += ef[:, 2:3] * W1[2] + ef[:, 3:4] * W1[3]
    h1 += b1
    np.maximum(h1, 0.0, out=h1)
    # pairwise error feedback: push the rounding residual of even edges into
    # the next (odd) edge of the same segment, so per-segment sums of the
    # quantized h1 are closer to exact.
    ev, od = h1[0::2], h1[1::2]
    qe = ev.astype(F8)
    res = ev - qe.astype(np.float32)
    same = (seg[0::2] == seg[1::2])
    qo = (od + res * same[:, None]).astype(F8)
    h1_8 = np.empty_like(h1, dtype=F8)            # [E, 256]
    h1_8[0::2] = qe
    h1_8[1::2] = qo

    w2h, w2l = _split_fp8(ws["Wp2"].reshape(2, 128, 256).transpose(1, 0, 2)
                          .reshape(128, 512))
    const_w = {
        "Wp2": np.concatenate([w2h, w2l], axis=1),
        "Wp3": ws["Wp3"].astype(ml_dtypes.bfloat16),
        "bp2": ws["bp2"].reshape(256, 1),
        "bp3": ws["bp3"].reshape(1, 64).astype(ml_dtypes.bfloat16),
        "ident": np.eye(128, dtype=np.float32),
        "ones": np.ones((1, AG_PER_CORE), np.float32),
        "Wr1": np.vstack([ws["Wr1"], ws["br1"].reshape(1, 256)]),
        "Wr2": ws["Wr2"],
        "br2": ws["br2"].reshape(1, 256),
        "Wr3": ws["Wr3"],
    }
    const_w = {k: np.ascontiguousarray(v) for k, v in const_w.items()}

    in_maps = []
    for i in range(N_CORES):
        h1h = np.zeros((128, NBLK, 2, E_BLK), F8)
        ohh = np.zeros((128, NBLK, C, 128), np.uint16)
        deg = np.zeros(AG_PER_CORE, np.float32)
        for j in range(NBLK):
            g = NBLK * i + j
            s, e = bounds[g], bounds[g + 1]
            cnt = e - s
            # [cnt, 256] -> [256, cnt] -> [2, 128, cnt] -> [128(p), 2(k), cnt]
            hb = np.ascontiguousarray(h1_8[s:e].T).reshape(2, 128, cnt)
            h1h[:, j, :, :cnt] = hb.transpose(1, 0, 2)
            ids = (seg[s:e] - 128 * g).astype(np.int64)
            oh2d = np.zeros((E_BLK, 128), np.uint16)
            oh2d[np.arange(cnt), ids] = 0x3F80    # 1.0 in bfloat16
            ohh[:, j] = oh2d.reshape(C, 128, 128).transpose(1, 0, 2)
            np.add.at(deg, seg[s:e] - AG_PER_CORE * i, 1.0)
        m = {"h1": h1h.reshape(128, -1),
             "oh": ohh.reshape(128, -1).view(ml_dtypes.bfloat16),
             "degT": deg.reshape(1, -1).astype(ml_dtypes.bfloat16)}
        m.update(const_w)
        in_maps.append(m)
    return C, in_maps


def _host_barrier(edge_feats, segment_ids):
    ef = np.asarray(edge_feats, dtype=np.float64)
    seg = np.asarray(segment_ids).astype(np.int64)
    p = ef[:, :2]
    d = np.sqrt((p * p).sum(1, keepdims=True))
    contrib = -(p / d) / (d - MARGIN)
    barrier = np.zeros((N_AGENTS, 2), np.float64)
    np.add.at(barrier, seg, contrib)
    return barrier


def kernel(edge_feats, segment_ids, Wp1, bp1, Wp2, bp2, Wp3, bp3,
           Wr1, br1, Wr2, br2, Wr3, br3, _trace=False):
    from concourse.bass_utils import run_bass_kernel_spmd

    ws = dict(Wp1=Wp1, bp1=bp1, Wp2=Wp2, bp2=bp2, Wp3=Wp3, bp3=bp3,
              Wr1=Wr1, br1=br1, Wr2=Wr2, br2=br2, Wr3=Wr3, br3=br3)
    ws = {k: np.asarray(v, dtype=np.float32) for k, v in ws.items()}
    C, in_maps = _prep_inputs(edge_feats, segment_ids, ws)
    if C not in _compiled:
        _compiled[C] = _build(C)
    nc = _compiled[C]
    res = run_bass_kernel_spmd(nc, in_maps, list(range(N_CORES)),
                               trace=_trace)
    outs = []
    for i in range(N_CORES):
        o = res.results[i]["out"]                 # [128, 32]
        outs.append(o.reshape(128, 4, 4, 2).transpose(1, 2, 0, 3)
                     .reshape(AG_PER_CORE, 2))
    out = np.concatenate(outs, 0)
    out = (out.astype(np.float64) + _host_barrier(edge_feats, segment_ids)
           + np.asarray(ws["br3"], np.float64).reshape(1, 2))
    if _trace:
        kernel._last_results = res
    return out.astype(np.float32)


# revision 19
# speedup vs baseline: 1.3100x; 1.3100x over previous
"""Trainium2 Bass kernel for nn_Barrier_Net (DeepSet GNN message passing).

Strategy (8 NeuronCores, SPMD):
  - Each core owns 2048 contiguous agents (16 blocks of 128 agents).
  - Host slices the sorted edge list at agent-range boundaries and pads each
    128-agent block to a uniform C chunks of 128 edges (identical program on
    every core; no collectives).
  - phi layer 1 (E x 4 @ 4 x 256, 0.5% of FLOPs) runs on the host; its
    ReLU output ships to the device as fp8e4m3 packed [128, 2, E] (the two
    128-feature halves on the k-axis) so phi layer 2 runs as K=256 fp8
    DoubleRow matmuls (0.5 cycles/row).
  - Weights Wp2/Wp3 use split-fp8 error compensation (W ~ hi + lo, two
    accumulating DoubleRow matmuls): weight quantization error is correlated
    across edges and does not average out in the segment sum, unlike the
    per-edge activation rounding noise.
  - phi layer 3 and the segment-sum (one-hot matmul; one-hot built on host,
    shipped as fp8 0/1) also run in fp8 DoubleRow.  PSUM stays fp32.
  - bp3 folds in as a rank-1 (degree x bp3) matmul; br1 via a constant-ones
    row appended to the aggregate; br2 via a rank-1 (br2 x ones) matmul.
    When bp2 is all-zero (checked at runtime) the h2 PSUM drain is a single
    merged bias-free ReLU instruction per chunk.
  - rho runs data-parallel over the agent dim in fp32r, interleaved with the
    edge blocks (group g is emitted after block 4g+3).
  - Only ACT/DVE can read PSUM (GPSIMD is SBUF-only), so the PSUM drains
    alternate between those two engines, weighted by their clocks.
  - The barrier term and br3 (negligible FLOPs) are added on the host.
"""

import numpy as np

N_AGENTS = 16384
N_EDGES = 524288
N_CORES = 8
AG_PER_CORE = N_AGENTS // N_CORES  # 2048
BLK = 128                          # agents per block
NBLK = AG_PER_CORE // BLK          # 16 blocks per core
MARGIN = 1.2 * 0.15                # barrier margin

A_BUFS = 8
DRAIN_SPLIT = False
PAIR_COPY = True
H1_BUFS = 3
OH_BUFS = 3
PS_MLP_BUFS = 2
PS_SM_BUFS = 2
PS_AGG_BUFS = 2

_compiled = {}


def _build(C, zb):
    """Build + schedule the SPMD Bass program for C 128-edge subchunks/block.

    zb: all phi biases zero -> merged (bias-free) h2 PSUM drains.
    The emission order is software-pipelined: for chunk index t we emit the
    L2 matmuls of chunk t, the PSUM drains of chunk t-1, and the
    L3/copy/aggregate stage of chunk t-2, so every engine's in-order
    instruction queue sees work whose dependencies are already in flight
    (queue depth on TRN2 is 4 per engine; emitting a chunk's full chain
    back-to-back serializes the machine to pipeline depth 1).
    """
    from collections import deque
    from contextlib import ExitStack

    import concourse.bass as bass
    import concourse.tile as tile
    from concourse import bacc, mybir

    FP = mybir.dt.float32
    RDT = mybir.dt.float32r
    BF = mybir.dt.bfloat16
    F8 = mybir.dt.float8e4
    DR = mybir.MatmulPerfMode.DoubleRow
    E_BLK = C * 128                     # padded edges per block

    nc = bacc.Bacc("TRN2", target_bir_lowering=False, debug=False,
                   num_devices=N_CORES)

    def din(name, shape, dt):
        return nc.dram_tensor(name, shape, dt, kind="ExternalInput").ap()

    h1d = din("h1", [128, NBLK * 2 * E_BLK], F8)
    ohd = din("oh", [128, NBLK * C * 128], BF)
    Wp2 = din("Wp2", [128, 1024], F8)       # hi || lo, each [128, 2, 256]
    Wp3 = din("Wp3", [256, 64], BF)
    bp2 = din("bp2", [256, 1], FP)
    bp3 = din("bp3", [1, 64], BF)
    degT = din("degT", [1, AG_PER_CORE], BF)
    ident = din("ident", [128, 128], FP)
    ones = din("ones", [1, AG_PER_CORE], RDT)
    Wr1 = din("Wr1", [65, 256], RDT)
    Wr2 = din("Wr2", [256, 256], RDT)
    br2 = din("br2", [1, 256], RDT)
    Wr3 = din("Wr3", [256, 2], FP)
    out_d = nc.dram_tensor("out", [128, 32], FP, kind="ExternalOutput").ap()

    RELU = mybir.ActivationFunctionType.Relu
    COPY = mybir.ActivationFunctionType.Copy
    ADD = mybir.AluOpType.add
    MAX = mybir.AluOpType.max

    with tile.TileContext(nc) as tc, ExitStack() as ctx:
        consts = ctx.enter_context(tc.tile_pool(name="consts", bufs=1))
        h1_pool = ctx.enter_context(tc.tile_pool(name="h1p", bufs=H1_BUFS))
        oh_pool = ctx.enter_context(tc.tile_pool(name="ohp", bufs=OH_BUFS))
        a_pool = ctx.enter_context(tc.tile_pool(name="acts", bufs=A_BUFS))
        ps_mlp = ctx.enter_context(
            tc.tile_pool(name="ps_mlp", bufs=PS_MLP_BUFS, space="PSUM"))
        ps_sm = ctx.enter_context(
            tc.tile_pool(name="ps_sm", bufs=PS_SM_BUFS, space="PSUM"))
        ps_agg = ctx.enter_context(
            tc.tile_pool(name="ps_agg", bufs=PS_AGG_BUFS, space="PSUM"))

        def cload(name, ap, shape=None, dt=FP):
            t = consts.tile(shape or list(ap.shape), dt, tag=name, name=name)
            nc.sync.dma_start(t[:], ap)
            return t

        wp2h_s = cload("wp2h", Wp2[:, 0:512], shape=[128, 2, 256], dt=F8)
        wp2l_s = cload("wp2l", Wp2[:, 512:1024], shape=[128, 2, 256], dt=F8)
        wp3a_s = cload("wp3a", Wp3[0:128, :], dt=BF)
        wp3b_s = cload("wp3b", Wp3[128:256, :], dt=BF)
        bp2a = cload("bp2a", bp2[0:128, :])
        bp2b = cload("bp2b", bp2[128:256, :])
        bp3_s = cload("bp3", bp3, dt=BF)
        degT_s = cload("degT", degT, dt=BF)
        ident_s = cload("ident", ident)
        wr1_s = cload("wr1", Wr1, dt=RDT)
        wr2a_s = cload("wr2a", Wr2[0:128, :], dt=RDT)
        wr2b_s = cload("wr2b", Wr2[128:256, :], dt=RDT)
        br2_s = cload("br2", br2, dt=RDT)
        wr3a_s = cload("wr3a", Wr3[0:128, :])
        wr3b_s = cload("wr3b", Wr3[128:256, :])
        ones_s = cload("ones_s", ones, dt=RDT)
        aggT_s = consts.tile([65, AG_PER_CORE], RDT, tag="aggT", name="aggT")
        nc.sync.dma_start(aggT_s[64:65, :], ones)
        osb = consts.tile([128, 32], FP, tag="osb", name="osb")

        # elementwise helpers; only ACT(0)/DVE(1) may read PSUM ----------
        def e_relu(eng, out, in_, bias=None):
            if eng == 0:
                nc.scalar.activation(out, in_, RELU,
                                     bias=0.0 if bias is None else bias)
            else:
                if bias is None:
                    nc.vector.tensor_scalar(out, in_, 0.0, None, MAX)
                else:
                    nc.vector.tensor_scalar(out, in_, bias, 0.0, ADD, MAX)

        def e_copy(eng, out, in_):
            if eng == 0:
                nc.scalar.activation(out, in_, COPY)
            else:
                nc.vector.tensor_copy(out, in_)

        # chunk sizes per block: C//4 chunks of 512 edges + one tail
        sizes = [512] * (C // 4)
        if C % 4:
            sizes.append(128 * (C % 4))

        chunks = []
        for j in range(NBLK):
            off = 0
            s0 = 0
            for idx, n in enumerate(sizes):
                # pair consecutive full chunks so one PSUM->SBUF copy serves
                # two chunks of L3 output (pair slot 0/1; tail alone)
                if PAIR_COPY and idx % 2 == 1 and n == 512 and \
                        sizes[idx - 1] == 512:
                    pslot = 1
                elif PAIR_COPY and idx + 1 < len(sizes) and n == 512 and \
                        sizes[idx + 1] == 512 and idx % 2 == 0:
                    pslot = 0
                else:
                    pslot = None
                chunks.append(dict(j=j, off=off, n=n, s0=s0,
                                   first=(idx == 0),
                                   last=(idx == len(sizes) - 1),
                                   pslot=pslot))
                off += n
                s0 += n // 128
        T = len(chunks)

        # engine pattern, period 5: 3/5 of merged drains on ACT (1.2 GHz),
        # 2/5 on DVE (0.96 GHz); the h3 copy goes to the other engine.
        DE = [0, 1, 0, 1, 0]

        blk = {}          # j -> [h1_blk, oh_blk, pagg, agg_first]
        ps2_t, h2_t, ps3_t = {}, {}, {}
        pending = deque()  # staged rho callables

        def emit_dma(j):
            if j >= NBLK:
                return
            h1_blk = h1_pool.tile([128, 2, E_BLK], F8, tag="h1",
                                  name=f"h1b{j}")
            nc.sync.dma_start(h1_blk[:],
                              h1d[:, j * 2 * E_BLK:(j + 1) * 2 * E_BLK])
            oh_blk = oh_pool.tile([128, C, 128], BF, tag="oh",
                                  name=f"ohb{j}")
            nc.sync.dma_start(oh_blk[:],
                              ohd[:, j * C * 128:(j + 1) * C * 128])
            blk[j] = [h1_blk, oh_blk, None, True]

        def stage_mm2(c):
            ch = chunks[c]
            if ch["first"]:
                emit_dma(ch["j"] + 1)
            n, off = ch["n"], ch["off"]
            h1sl = blk[ch["j"]][0][:, :, off:off + n]
            ps2 = ps_mlp.tile([128, 2, 512], FP, tag="psmlp", name=f"ps2_{c}")
            ps2_t[c] = ps2
            nc.tensor.matmul(ps2[:, 0, 0:n], wp2h_s[:, :, 0:128], h1sl,
                             start=True, stop=False, perf_mode=DR)
            nc.tensor.matmul(ps2[:, 0, 0:n], wp2l_s[:, :, 0:128], h1sl,
                             start=False, stop=True, perf_mode=DR)
            nc.tensor.matmul(ps2[:, 1, 0:n], wp2h_s[:, :, 128:256], h1sl,
                             start=True, stop=False, perf_mode=DR)
            nc.tensor.matmul(ps2[:, 1, 0:n], wp2l_s[:, :, 128:256], h1sl,
                             start=False, stop=True, perf_mode=DR)

        def stage_drain(c):
            ch = chunks[c]
            n = ch["n"]
            de = DE[c % 5]
            ps2 = ps2_t.pop(c)
            h2 = a_pool.tile([128, 2, 512], BF, tag="h2", name=f"h2_{c}")
            h2_t[c] = h2
            if zb and not DRAIN_SPLIT:
                e_relu(de, h2[:, :, 0:n], ps2[:, :, 0:n])
            elif zb:
                e_relu(de, h2[:, 0, 0:n], ps2[:, 0, 0:n])
                e_relu(1 - de, h2[:, 1, 0:n], ps2[:, 1, 0:n])
            else:
                e_relu(de, h2[:, 0, 0:n], ps2[:, 0, 0:n], bias=bp2a[:, 0:1])
                e_relu(1 - de, h2[:, 1, 0:n], ps2[:, 1, 0:n],
                       bias=bp2b[:, 0:1])

        pair_res = {}
        fin_res = {}

        def stage_l3(c):
            ch = chunks[c]
            j, n = ch["j"], ch["n"]
            nsub = n // 128
            h2 = h2_t.pop(c)
            pslot = ch["pslot"]
            if pslot in (None, 0):
                ps3 = ps_sm.tile([128, 8, 64], FP, tag="sm",
                                 name=f"ps3_{c}")
                h3 = a_pool.tile([128, 8, 64], BF, tag="h3",
                                 name=f"h3_{c}")
                if pslot == 0:
                    pair_res[j] = (ps3, h3)
            else:
                ps3, h3 = pair_res.pop(j)
            base = 4 * (pslot == 1)
            for s in range(nsub):
                ssl = slice(s * 128, (s + 1) * 128)
                nc.tensor.matmul(ps3[:, base + s, :], h2[:, 0, ssl],
                                 wp3a_s[:], start=True, stop=False)
                nc.tensor.matmul(ps3[:, base + s, :], h2[:, 1, ssl],
                                 wp3b_s[:], start=False, stop=True)
            fin_res[c] = (ps3, h3)

        def stage_fin(c):
            ch = chunks[c]
            j, n, s0 = ch["j"], ch["n"], ch["s0"]
            nsub = n // 128
            ce = 1 - DE[c % 5]
            pslot = ch["pslot"]
            ps3, h3 = fin_res.pop(c)
            b = blk[j]
            if b[2] is None:
                b[2] = ps_agg.tile([128, 64], FP, tag="agg", name=f"pagg{j}")
            pagg = b[2]
            oh_blk = b[1]
            if pslot == 0:
                pass
            elif pslot is None:
                e_copy(ce, h3[:, 0:nsub, :], ps3[:, 0:nsub, :])
                for t in range(nsub):
                    nc.tensor.matmul(pagg[:], oh_blk[:, s0 + t, :],
                                     h3[:, t, :], start=b[3], stop=False)
                    b[3] = False
            else:
                e_copy(ce, h3[:, :, :], ps3[:, :, :])
                ps0 = s0 - 4
                for t in range(8):
                    nc.tensor.matmul(pagg[:], oh_blk[:, ps0 + t, :],
                                     h3[:, t, :], start=b[3], stop=False)
                    b[3] = False
            if ch["last"]:
                nc.tensor.matmul(pagg[:], degT_s[:, j * 128:(j + 1) * 128],
                                 bp3_s[:], start=b[3], stop=True)
                queue_tail(j, pagg)
                if j % 4 == 3:
                    queue_rho(j // 4)
            if pending:
                pending.popleft()()

        def queue_tail(j, pagg):
            st = {}

            def tail_a():
                agg_sb = a_pool.tile([128, 64], FP, tag="aggsb",
                                     name=f"asb{j}")
                st["sb"] = agg_sb
                e_copy(1, agg_sb[:], pagg[:])
                pst = ps_sm.tile([64, 128], FP, tag="sm", name=f"pst{j}")
                st["pst"] = pst
                nc.tensor.transpose(pst[:], agg_sb[:], ident_s[:])

            def tail_b():
                e_copy(j % 2, aggT_s[0:64, j * 128:(j + 1) * 128],
                       st["pst"])

            pending.extend([tail_a, tail_b])

        def queue_rho(g):
            sl = slice(g * 512, (g + 1) * 512)
            st = {}

            def rho_a():
                pr1 = ps_mlp.tile([128, 2, 512], FP, tag="psmlp",
                                  name=f"pr1_{g}")
                st["pr1"] = pr1
                nc.tensor.matmul(pr1[:, 0, :], wr1_s[:, 0:128],
                                 aggT_s[:, sl], start=True, stop=True)
                nc.tensor.matmul(pr1[:, 1, :], wr1_s[:, 128:256],
                                 aggT_s[:, sl], start=True, stop=True)

            def rho_b1():
                pr1 = st["pr1"]
                r1 = a_pool.tile([128, 2, 512], RDT, tag="big",
                                 name=f"r1_{g}")
                st["r1"] = r1
                e_relu(g % 2, r1[:, :, :], pr1[:, :, :])
                pr2 = ps_mlp.tile([128, 2, 512], FP, tag="psmlp",
                                  name=f"pr2_{g}")
                st["pr2"] = pr2
                nc.tensor.matmul(pr2[:, 0, :], wr2a_s[:, 0:128], r1[:, 0, :],
                                 start=True, stop=False)
                nc.tensor.matmul(pr2[:, 0, :], wr2b_s[:, 0:128], r1[:, 1, :],
                                 start=False, stop=False)
                nc.tensor.matmul(pr2[:, 0, :], br2_s[0:1, 0:128],
                                 ones_s[0:1, sl], start=False, stop=True)

            def rho_b2():
                pr2, r1 = st["pr2"], st["r1"]
                nc.tensor.matmul(pr2[:, 1, :], wr2a_s[:, 128:256],
                                 r1[:, 0, :], start=True, stop=False)
                nc.tensor.matmul(pr2[:, 1, :], wr2b_s[:, 128:256],
                                 r1[:, 1, :], start=False, stop=False)
                nc.tensor.matmul(pr2[:, 1, :], br2_s[0:1, 128:256],
                                 ones_s[0:1, sl], start=False, stop=True)

            def rho_c1():
                pr2 = st["pr2"]
                r2 = a_pool.tile([128, 2, 512], FP, tag="big",
                                 name=f"r2_{g}")
                st["r2"] = r2
                e_relu(1 - g % 2, r2[:, :, :], pr2[:, :, :])

            def rho_c2():
                r2 = st["r2"]
                pso = ps_sm.tile([128, 8], FP, tag="sm", name=f"pso{g}")
                for s in range(4):
                    ssl = slice(s * 128, (s + 1) * 128)
                    nc.tensor.matmul(pso[:, s * 2:s * 2 + 2], r2[:, 0, ssl],
                                     wr3a_s[:], start=True, stop=False)
                    nc.tensor.matmul(pso[:, s * 2:s * 2 + 2], r2[:, 1, ssl],
                                     wr3b_s[:], start=False, stop=True)
                e_copy(g % 2, osb[:, g * 8:(g + 1) * 8], pso[:])

            pending.extend([rho_a, rho_b1, rho_b2, rho_c1, rho_c2])

        emit_dma(0)
        for t in range(T + 3):
            if t < T:
                stage_mm2(t)
            if t >= 1 and t - 1 < T:
                stage_drain(t - 1)
            if t >= 2 and t - 2 < T:
                stage_l3(t - 2)
            if t >= 3:
                stage_fin(t - 3)
        while pending:
            pending.popleft()()

        nc.sync.dma_start(out_d, osb[:])

    nc.compile()
    return nc


def _split_fp8(W):
    import ml_dtypes
    F8 = ml_dtypes.float8_e4m3
    hi = W.astype(F8)
    lo = (W - hi.astype(np.float32)).astype(F8)
    return hi, lo


def _prep_inputs(edge_feats, segment_ids, ws):
    """Host-side: phi layer 1, one-hot build, shard + pad.

    Returns (C, zb, in_maps)."""
    import ml_dtypes
    F8 = ml_dtypes.float8_e4m3

    seg = np.asarray(segment_ids).astype(np.int64)
    ef = np.asarray(edge_feats, dtype=np.float32)
    bounds = np.searchsorted(seg, np.arange(0, N_AGENTS + 1, BLK))
    counts = np.diff(bounds)                      # edges per 128-agent block
    C = int(np.ceil(counts.max() / 128))
    E_BLK = C * 128
    zb = bool(np.all(ws["bp2"] == 0.0))

    # phi layer 1 on host (0.5% of FLOPs): h1 = relu(x @ Wp1 + bp1)
    W1, b1 = ws["Wp1"], ws["bp1"]
    h1 = ef[:, 0:1] * W1[0] + ef[:, 1:2] * W1[1]
    h1 += ef[:, 2:3] * W1[2] + ef[:, 3:4] * W1[3]
    h1 += b1
    np.maximum(h1, 0.0, out=h1)
    # pairwise error feedback: push the rounding residual of even edges into
    # the next (odd) edge of the same segment, so per-segment sums of the
    # quantized h1 are closer to exact.
    ev, od = h1[0::2], h1[1::2]
    qe = ev.astype(F8)
    res = ev - qe.astype(np.float32)
    same = (seg[0::2] == seg[1::2])
    qo = (od + res * same[:, None]).astype(F8)
    h1_8 = np.empty_like(h1, dtype=F8)            # [E, 256]
    h1_8[0::2] = qe
    h1_8[1::2] = qo

    w2h, w2l = _split_fp8(ws["Wp2"].reshape(2, 128, 256).transpose(1, 0, 2)
                          .reshape(128, 512))
    const_w = {
        "Wp2": np.concatenate([w2h, w2l], axis=1),
        "Wp3": ws["Wp3"].astype(ml_dtypes.bfloat16),
        "bp2": ws["bp2"].reshape(256, 1),
        "bp3": ws["bp3"].reshape(1, 64).astype(ml_dtypes.bfloat16),
        "ident": np.eye(128, dtype=np.float32),
        "ones": np.ones((1, AG_PER_CORE), np.float32),
        "Wr1": np.vstack([ws["Wr1"], ws["br1"].reshape(1, 256)]),
        "Wr2": ws["Wr2"],
        "br2": ws["br2"].reshape(1, 256),
        "Wr3": ws["Wr3"],
    }
    const_w = {k: np.ascontiguousarray(v) for k, v in const_w.items()}

    in_maps = []
    for i in range(N_CORES):
        h1h = np.zeros((128, NBLK, 2, E_BLK), F8)
        ohh = np.zeros((128, NBLK, C, 128), np.uint16)
        deg = np.zeros(AG_PER_CORE, np.float32)
        for j in range(NBLK):
            g = NBLK * i + j
            s, e = bounds[g], bounds[g + 1]
            cnt = e - s
            # [cnt, 256] -> [256, cnt] -> [2, 128, cnt] -> [128(p), 2(k), cnt]
            hb = np.ascontiguousarray(h1_8[s:e].T).reshape(2, 128, cnt)
            h1h[:, j, :, :cnt] = hb.transpose(1, 0, 2)
            ids = (seg[s:e] - 128 * g).astype(np.int64)
            oh2d = np.zeros((E_BLK, 128), np.uint16)
            oh2d[np.arange(cnt), ids] = 0x3F80    # 1.0 in bfloat16
            ohh[:, j] = oh2d.reshape(C, 128, 128).transpose(1, 0, 2)
            np.add.at(deg, seg[s:e] - AG_PER_CORE * i, 1.0)
        m = {"h1": h1h.reshape(128, -1),
             "oh": ohh.reshape(128, -1).view(ml_dtypes.bfloat16),
             "degT": deg.reshape(1, -1).astype(ml_dtypes.bfloat16)}
        m.update(const_w)
        in_maps.append(m)
    return C, zb, in_maps


def _host_barrier(edge_feats, segment_ids):
    ef = np.asarray(edge_feats, dtype=np.float64)
    seg = np.asarray(segment_ids).astype(np.int64)
    p = ef[:, :2]
    d = np.sqrt((p * p).sum(1, keepdims=True))
    contrib = -(p / d) / (d - MARGIN)
    barrier = np.zeros((N_AGENTS, 2), np.float64)
    np.add.at(barrier, seg, contrib)
    return barrier


def kernel(edge_feats, segment_ids, Wp1, bp1, Wp2, bp2, Wp3, bp3,
           Wr1, br1, Wr2, br2, Wr3, br3, _trace=False):
    from concourse.bass_utils import run_bass_kernel_spmd

    ws = dict(Wp1=Wp1, bp1=bp1, Wp2=Wp2, bp2=bp2, Wp3=Wp3, bp3=bp3,
              Wr1=Wr1, br1=br1, Wr2=Wr2, br2=br2, Wr3=Wr3, br3=br3)
    ws = {k: np.asarray(v, dtype=np.float32) for k, v in ws.items()}
    C, zb, in_maps = _prep_inputs(edge_feats, segment_ids, ws)
    key = (C, zb)
    if key not in _compiled:
        _compiled[key] = _build(C, zb)
    nc = _compiled[key]
    res = run_bass_kernel_spmd(nc, in_maps, list(range(N_CORES)),
                               trace=_trace)
    outs = []
    for i in range(N_CORES):
        o = res.results[i]["out"]                 # [128, 32]
        outs.append(o.reshape(128, 4, 4, 2).transpose(1, 2, 0, 3)
                     .reshape(AG_PER_CORE, 2))
    out = np.concatenate(outs, 0)
    out = (out.astype(np.float64) + _host_barrier(edge_feats, segment_ids)
           + np.asarray(ws["br3"], np.float64).reshape(1, 2))
    if _trace:
        kernel._last_results = res
    return out.astype(np.float32)
